# revision 8
# baseline (speedup 1.0000x reference)
"""Trainium2 Bass kernel for HF Open-MoE top-2 router (T=8192, E=64, cap=320).

Sharding: tokens split across 8 cores (1024 each), GShard-style. The only
cross-shard dependency is per-expert cumulative counts; the host performs that
tiny exchange (equivalent of the count all-gather) and feeds each core its
per-expert rank offsets. Each core computes softmax/top-2/gates/ranks for its
tokens on-device and materializes its dense [1024, 64, 320] slices of
cb_weight (f32) and sec_mask (u8) in HBM: bulk zero-fill DMAs from an SBUF
zero block + indirect-DMA scatter of the <=2 nonzeros per token.
"""

import os
import sys
import math

import numpy as np

sys.path.insert(0, "/opt/trn_rl_repo")

T, E, CAP = 8192, 64, 320
CORES = 8
TL = T // CORES          # 1024 tokens per core
P = 128                  # partitions
NCH = TL // P            # 8 chunks of 128 tokens per core
W = E * CAP              # 20480 = flattened (expert, slot) row width

_PROGRAM = None


def _build_program():
    """Build the per-core SPMD Bass program (same program on all 8 cores)."""
    import concourse.bass as bass
    import concourse.tile as tile
    from concourse import bacc, mybir

    fp32 = mybir.dt.float32
    i32 = mybir.dt.int32
    u32 = mybir.dt.uint32
    u8 = mybir.dt.uint8
    Alu = mybir.AluOpType
    Act = mybir.ActivationFunctionType

    nc = bacc.Bacc(
        "TRN2",
        target_bir_lowering=False,
        debug=False,
        enable_asserts=True,
        num_devices=1,
    )

    # ---- DRAM I/O ----
    x_d = nc.dram_tensor("x", [TL, E], fp32, kind="ExternalInput")
    off1_d = nc.dram_tensor("off1", [1, E], fp32, kind="ExternalInput")
    off2_d = nc.dram_tensor("off2", [1, E], fp32, kind="ExternalInput")
    tri_d = nc.dram_tensor("tri", [P, P], fp32, kind="ExternalInput")
    ones_d = nc.dram_tensor("ones1", [1, P], fp32, kind="ExternalInput")
    cb_d = [
        nc.dram_tensor(f"cb{ch}", [P * W, 1], fp32, kind="ExternalOutput")
        for ch in range(NCH)
    ]
    sec_d = [
        nc.dram_tensor(f"sec{ch}", [P * W, 1], u8, kind="ExternalOutput")
        for ch in range(NCH)
    ]

    with tile.TileContext(nc) as tc:
        with (
            tc.tile_pool(name="static", bufs=1) as static,
            tc.tile_pool(name="work", bufs=3) as work,
            tc.tile_pool(name="small", bufs=4) as small,
            tc.tile_pool(name="psum", bufs=2, space="PSUM") as psum,
        ):
            # ---- static tiles ----
            zeros_f = static.tile([P, W], fp32)       # 10 MB zero source (cb)
            zeros_b = static.tile([P, W], u8)         # 2.5 MB zero source (sec)
            nc.vector.memset(zeros_f[:], 0.0)
            nc.gpsimd.memset(zeros_b[:], 0)

            tri_s = static.tile([P, P], fp32)
            ones_s = static.tile([1, P], fp32)
            ones_c = static.tile([P, 1], fp32)
            nc.scalar.dma_start(tri_s[:], tri_d.ap())
            nc.scalar.dma_start(ones_s[:], ones_d.ap())
            nc.vector.memset(ones_c[:], 1.0)

            iota_e = static.tile([P, E], fp32)         # [p, e] = e
            nc.gpsimd.iota(
                iota_e[:], pattern=[[1, E]], base=0, channel_multiplier=0,
                allow_small_or_imprecise_dtypes=True,
            )
            tokbase = static.tile([P, 1], i32)         # [p, 0] = p * W
            nc.gpsimd.iota(tokbase[:], pattern=[[0, 1]], base=0, channel_multiplier=W)

            carry1 = static.tile([1, E], fp32)
            carry2 = static.tile([1, E], fp32)
            nc.scalar.dma_start(carry1[:], off1_d.ap())
            nc.scalar.dma_start(carry2[:], off2_d.ap())

            # ---- bulk zero-fill of all outputs (independent of compute) ----
            for ch in range(NCH):
                cb_view = cb_d[ch].ap().rearrange("(p w) o -> p (w o)", p=P)
                sec_view = sec_d[ch].ap().rearrange("(p w) o -> p (w o)", p=P)
                nc.sync.dma_start(cb_view, zeros_f[:])
                nc.sync.dma_start(sec_view, zeros_b[:])

            # ---- per-chunk compute + scatter ----
            for ch in range(NCH):
                x_t = work.tile([P, E], fp32)
                nc.scalar.dma_start(x_t[:], x_d.ap()[ch * P:(ch + 1) * P, :])

                topv = small.tile([P, 8], fp32)
                topi = small.tile([P, 8], u32)
                nc.vector.max_with_indices(topv[:], topi[:], x_t[:])

                # softmax pieces: w1 = 1/sum(exp(x - v1)); w2 = exp(v2 - v1) * w1
                negv1 = small.tile([P, 1], fp32)
                nc.vector.tensor_scalar_mul(negv1[:], topv[:, 0:1], -1.0)
                exp_t = work.tile([P, E], fp32)
                sumexp = small.tile([P, 1], fp32)
                nc.scalar.activation(
                    exp_t[:], x_t[:], Act.Exp, bias=negv1[:], accum_out=sumexp[:]
                )
                w1 = small.tile([P, 1], fp32)
                nc.vector.reciprocal(w1[:], sumexp[:])
                d2 = small.tile([P, 1], fp32)
                nc.vector.tensor_tensor(
                    out=d2[:], in0=topv[:, 1:2], in1=topv[:, 0:1], op=Alu.subtract
                )
                e2 = small.tile([P, 1], fp32)
                nc.scalar.activation(e2[:], d2[:], Act.Exp)
                w2 = small.tile([P, 1], fp32)
                nc.vector.tensor_tensor(out=w2[:], in0=e2[:], in1=w1[:], op=Alu.mult)

                # one-hots of top1/top2 (compare scalars must be f32)
                k1f = small.tile([P, 1], fp32)
                k2f = small.tile([P, 1], fp32)
                nc.vector.tensor_copy(k1f[:], topi[:, 0:1])
                nc.vector.tensor_copy(k2f[:], topi[:, 1:2])
                oh1 = work.tile([P, E], fp32)
                oh2 = work.tile([P, E], fp32)
                nc.vector.tensor_scalar(
                    out=oh1[:], in0=iota_e[:], scalar1=k1f[:], scalar2=None,
                    op0=Alu.is_equal,
                )
                nc.vector.tensor_scalar(
                    out=oh2[:], in0=iota_e[:], scalar1=k2f[:], scalar2=None,
                    op0=Alu.is_equal,
                )

                # inclusive cumsum ranks via triangular matmul + carry row
                ps1 = psum.tile([P, E], fp32, space="PSUM")
                nc.tensor.matmul(ps1[:], lhsT=tri_s[:], rhs=oh1[:], start=True, stop=False)
                nc.tensor.matmul(ps1[:], lhsT=ones_s[:], rhs=carry1[:], start=False, stop=True)
                ps2 = psum.tile([P, E], fp32, space="PSUM")
                nc.tensor.matmul(ps2[:], lhsT=tri_s[:], rhs=oh2[:], start=True, stop=False)
                nc.tensor.matmul(ps2[:], lhsT=ones_s[:], rhs=carry2[:], start=False, stop=True)

                # carry += per-chunk expert totals (ones-column matmul colsum)
                tot1 = psum.tile([1, E], fp32, space="PSUM")
                tot2 = psum.tile([1, E], fp32, space="PSUM")
                nc.tensor.matmul(tot1[:], lhsT=ones_c[:], rhs=oh1[:], start=True, stop=True)
                nc.tensor.matmul(tot2[:], lhsT=ones_c[:], rhs=oh2[:], start=True, stop=True)
                nxt1 = static.tile([1, E], fp32, tag=f"carry1_{ch}")
                nxt2 = static.tile([1, E], fp32, tag=f"carry2_{ch}")
                nc.vector.tensor_tensor(out=nxt1[:], in0=carry1[:], in1=tot1[:], op=Alu.add)
                nc.vector.tensor_tensor(out=nxt2[:], in0=carry2[:], in1=tot2[:], op=Alu.add)
                carry1, carry2 = nxt1, nxt2

                # per-token inclusive rank (r+1) = rowsum(psum * onehot)
                junk1 = work.tile([P, E], fp32)
                junk2 = work.tile([P, E], fp32)
                r1i = small.tile([P, 1], fp32)
                r2i = small.tile([P, 1], fp32)
                nc.vector.scalar_tensor_tensor(
                    out=junk1[:], in0=ps1[:], scalar=1.0, in1=oh1[:],
                    op0=Alu.mult, op1=Alu.mult, accum_out=r1i[:],
                )
                nc.vector.scalar_tensor_tensor(
                    out=junk2[:], in0=ps2[:], scalar=1.0, in1=oh2[:],
                    op0=Alu.mult, op1=Alu.mult, accum_out=r2i[:],
                )

                # keep flags, clamped slot, flat index
                vals_cb = small.tile([P, 2], fp32)
                vals_sec = small.tile([P, 2], u8)
                offs = small.tile([P, 2], i32)
                kept1 = small.tile([P, 1], fp32)
                kept2 = small.tile([P, 1], fp32)
                nc.vector.tensor_scalar(
                    out=kept1[:], in0=r1i[:], scalar1=float(CAP), scalar2=None,
                    op0=Alu.is_le,
                )
                nc.vector.tensor_scalar(
                    out=kept2[:], in0=r2i[:], scalar1=float(CAP), scalar2=None,
                    op0=Alu.is_le,
                )
                # gate values (0 if dropped)
                nc.vector.tensor_tensor(
                    out=vals_cb[:, 0:1], in0=w1[:], in1=kept1[:], op=Alu.mult
                )
                nc.vector.tensor_tensor(
                    out=vals_cb[:, 1:2], in0=w2[:], in1=kept2[:], op=Alu.mult
                )
                nc.vector.tensor_copy(vals_sec[:, 0:1], kept1[:])
                nc.vector.tensor_copy(vals_sec[:, 1:2], kept2[:])

                # slot = min(r_inclusive, CAP) - 1  in [0, CAP-1]
                r1c = small.tile([P, 1], fp32)
                r2c = small.tile([P, 1], fp32)
                nc.vector.tensor_scalar(
                    out=r1c[:], in0=r1i[:], scalar1=float(CAP), scalar2=1.0,
                    op0=Alu.min, op1=Alu.subtract,
                )
                nc.vector.tensor_scalar(
                    out=r2c[:], in0=r2i[:], scalar1=float(CAP), scalar2=1.0,
                    op0=Alu.min, op1=Alu.subtract,
                )
                # flat idx = e * CAP + slot (exact in f32), then int add p*W
                idx1f = small.tile([P, 1], fp32)
                idx2f = small.tile([P, 1], fp32)
                nc.vector.tensor_scalar(
                    out=idx1f[:], in0=k1f[:], scalar1=float(CAP), scalar2=r1c[:],
                    op0=Alu.mult, op1=Alu.add,
                )
                nc.vector.tensor_scalar(
                    out=idx2f[:], in0=k2f[:], scalar1=float(CAP), scalar2=r2c[:],
                    op0=Alu.mult, op1=Alu.add,
                )
                idx1i = small.tile([P, 1], i32)
                idx2i = small.tile([P, 1], i32)
                nc.vector.tensor_copy(idx1i[:], idx1f[:])
                nc.vector.tensor_copy(idx2i[:], idx2f[:])
                nc.vector.tensor_tensor(
                    out=offs[:, 0:1], in0=idx1i[:], in1=tokbase[:], op=Alu.add
                )
                nc.vector.tensor_tensor(
                    out=offs[:, 1:2], in0=idx2i[:], in1=tokbase[:], op=Alu.add
                )

                # scatter the nonzeros over the zero-filled chunks.
                # NB: HW indirect DMA consumes ONE offset per partition row
                # (writes the row contiguously from it), so scatter each
                # element with its own [P, 1] call.
                for j in range(2):
                    nc.gpsimd.indirect_dma_start(
                        out=cb_d[ch].ap(),
                        out_offset=bass.IndirectOffsetOnAxis(
                            ap=offs[:, j:j + 1], axis=0
                        ),
                        in_=vals_cb[:, j:j + 1],
                        in_offset=None,
                    )
                    nc.gpsimd.indirect_dma_start(
                        out=sec_d[ch].ap(),
                        out_offset=bass.IndirectOffsetOnAxis(
                            ap=offs[:, j:j + 1], axis=0
                        ),
                        in_=vals_sec[:, j:j + 1],
                        in_offset=None,
                    )

    nc.compile()
    return nc


def _get_program():
    global _PROGRAM
    if _PROGRAM is None:
        _PROGRAM = _build_program()
    return _PROGRAM


def _host_prepass(x: np.ndarray):
    """Routing counts + cross-shard offsets (the host-side 'collective')."""
    top1 = np.argmax(x, axis=1)
    xm = x.copy()
    xm[np.arange(T), top1] = -np.inf
    top2 = np.argmax(xm, axis=1)
    cnt1 = np.stack(
        [np.bincount(top1[c * TL:(c + 1) * TL], minlength=E) for c in range(CORES)]
    )
    cnt2 = np.stack(
        [np.bincount(top2[c * TL:(c + 1) * TL], minlength=E) for c in range(CORES)]
    )
    off1 = (np.cumsum(cnt1, axis=0) - cnt1).astype(np.float32)
    t1 = cnt1.sum(axis=0)
    t2 = cnt2.sum(axis=0)
    off2 = ((np.cumsum(cnt2, axis=0) - cnt2) + t1[None, :]).astype(np.float32)
    used_capacity = (
        np.minimum(t1, CAP) + np.clip(CAP - t1, 0, t2)
    ).astype(np.int32)
    return off1, off2, used_capacity


def _make_in_maps(x: np.ndarray, off1: np.ndarray, off2: np.ndarray):
    tri = np.tril(np.ones((P, P), dtype=np.float32)).T.copy()  # tri[k, m] = k <= m
    ones1 = np.ones((1, P), dtype=np.float32)
    in_maps = []
    for c in range(CORES):
        in_maps.append(
            {
                "x": np.ascontiguousarray(x[c * TL:(c + 1) * TL, :]),
                "off1": off1[c:c + 1, :],
                "off2": off2[c:c + 1, :],
                "tri": tri,
                "ones1": ones1,
            }
        )
    return in_maps


def kernel(inputs: np.ndarray):
    from concourse.bass_utils import run_bass_kernel_spmd

    x = np.asarray(inputs, dtype=np.float32)
    assert x.shape == (T, E)

    off1, off2, used_capacity = _host_prepass(x)
    nc = _get_program()
    in_maps = _make_in_maps(x, off1, off2)
    res = run_bass_kernel_spmd(nc, in_maps, core_ids=list(range(CORES)))

    cb = np.empty((T, E, CAP), dtype=np.float32)
    sec = np.empty((T, E, CAP), dtype=np.uint8)
    for c in range(CORES):
        out = res.results[c]
        for ch in range(NCH):
            sl = slice(c * TL + ch * P, c * TL + (ch + 1) * P)
            cb[sl] = out[f"cb{ch}"].reshape(P, E, CAP)
            sec[sl] = out[f"sec{ch}"].reshape(P, E, CAP)
    return used_capacity, cb, sec.view(np.bool_)
